# revision 1
# baseline (speedup 1.0000x reference)
"""Trainium2 Bass kernel for nn_Gemma3MoEAttention (B=4,T=2048,D=2048,NH=8,NKV=4,HD=256).

Self-contained: kernel(**inputs) takes the full unsharded inputs, shards across
8 NeuronCores (batch x query-block-halves, causal-balanced via the mask),
runs a float32r Bass/Tile kernel via run_bass_kernel_spmd, and gathers the
full [4,2048,2048] float32 output.
"""
"""Gemma3 MoE attention TRN2 kernel: builder + host-side sharding glue.

Sharding: 8 cores; core c -> batch b = c//2, plus 8 of the 16 query blocks of
128 rows (split by causal need so both cores of a pair do equal work). Each
core computes K/V for its full batch, Q for its blocks, flash attention with
the mask applied from data (block-sparse: per-slot kv extent from the mask),
and the out-projection for its rows. Host only slices/gathers.

All matmuls run in float32r (full PE speed, ~1.5e-4 rms error).
"""
import numpy as np
import ml_dtypes

import concourse.bass as bass
import concourse.mybir as mybir
import concourse.tile as tile
from concourse import bacc
from concourse.masks import make_identity

F32 = mybir.dt.float32
F32R = mybir.dt.float32r
BF16 = mybir.dt.bfloat16

B, T, D = 4, 2048, 2048
NH, NKV, HD = 8, 4, 256
G = NH // NKV
P = 128
NBLK = T // P          # 16 key/query blocks per batch
NSLOT = 8              # query blocks per core
NQ = NSLOT * P         # 1024 q rows per core
DC = D // P            # 16 contraction chunks
ROPE_BASE = 10000.0
SOFT_CAP = 50.0
EXP_SCALE = SOFT_CAP / float(np.sqrt(HD))   # 3.125
MASK_NEG = -60.0
AX = mybir.AxisListType
ALU = mybir.AluOpType
ACTF = mybir.ActivationFunctionType


def build_nc(slot_L, slot_P=None):
    """slot_L: per-slot kv extent in blocks of 128 (len 8, each even, 2..16).
    slot_P: per-slot count of leading kv-blocks that are all-True on every
    core (mask add skipped there); defaults to 0 (mask everything)."""
    if slot_P is None:
        slot_P = [0] * NSLOT
    assert len(slot_L) == NSLOT
    assert all(2 <= L <= NBLK and L % 2 == 0 for L in slot_L)
    assert all(0 <= p <= L for p, L in zip(slot_P, slot_L))
    nc = bacc.Bacc("TRN2", target_bir_lowering=False, debug=False, num_devices=8)

    xT = nc.declare_dram_parameter("xT", [D, T], F32R, isOutput=False)
    xqT = nc.declare_dram_parameter("xqT", [D, NQ], F32R, isOutput=False)
    wq = nc.declare_dram_parameter("wq", [D, NH * HD], F32R, isOutput=False)
    wk = nc.declare_dram_parameter("wk", [D, NKV * HD], F32R, isOutput=False)
    wv = nc.declare_dram_parameter("wv", [D, NKV * HD], F32R, isOutput=False)
    wo = nc.declare_dram_parameter("wo", [D, D], F32R, isOutput=False)
    cos_k = nc.declare_dram_parameter("cos_k", [P, T], F32, isOutput=False)
    sin_k = nc.declare_dram_parameter("sin_k", [P, T], F32, isOutput=False)
    cos_q = nc.declare_dram_parameter("cos_q", [P, NQ], F32, isOutput=False)
    sin_q = nc.declare_dram_parameter("sin_q", [P, NQ], F32, isOutput=False)
    maskneg = nc.declare_dram_parameter("maskneg", [NQ, T], BF16, isOutput=False)
    out = nc.declare_dram_parameter("out", [NQ, D], F32, isOutput=True)

    # DRAM scratch
    kT_s = nc.dram_tensor("kT_s", [NKV * HD, T], F32R)
    v_s = nc.dram_tensor("v_s", [T, NKV * HD], F32R)
    qT_s = nc.dram_tensor("qT_s", [NH * HD, NQ], F32R)

    a = dict(
        xT_r=xT.rearrange("(o p) t -> p o t", p=P),
        xqT_r=xqT.rearrange("(o p) t -> p o t", p=P),
        wq_r=wq.rearrange("(o p) c -> p o c", p=P),
        wk_r=wk.rearrange("(o p) c -> p o c", p=P),
        wv_r=wv.rearrange("(o p) c -> p o c", p=P),
        wo_r=wo.rearrange("(o p) c -> p o c", p=P),
        kT_sr=kT_s.rearrange("(o p) t -> p o t", p=P),
        qT_sr=qT_s.rearrange("(o p) t -> p o t", p=P),
        v_sr=v_s.rearrange("(o p) c -> p o c", p=P),
        mask_r=maskneg.rearrange("(s p) t -> p s t", p=P),
        out_r=out.rearrange("(s p) d -> p s d", p=P),
        cos_k=cos_k, sin_k=sin_k, cos_q=cos_q, sin_q=sin_q,
    )

    with tile.TileContext(nc) as tc:
        _emit_body(nc, tc, a, slot_L, slot_P)
    nc.finalize()
    return nc


def _rope_pair(nc, pool, ps0, ps1, cos_ap, sin_ap, dst0, dst1):
    """dst0 = ps0*cos - ps1*sin ; dst1 = ps1*cos + ps0*sin (DMA'd to dram)."""
    tA = pool.tile([P, 512], F32, tag="ropeA")
    nc.vector.tensor_tensor(tA[:], ps0, cos_ap, ALU.mult)
    tB = pool.tile([P, 512], F32, tag="ropeB")
    nc.vector.tensor_tensor(tB[:], ps1, sin_ap, ALU.mult)
    o1 = pool.tile([P, 512], F32R, tag="ropeO")
    nc.vector.tensor_tensor(o1[:], tA[:], tB[:], ALU.subtract)
    nc.sync.dma_start(dst0, o1[:])
    tC = pool.tile([P, 512], F32, tag="ropeA")
    nc.vector.tensor_tensor(tC[:], ps1, cos_ap, ALU.mult)
    tD = pool.tile([P, 512], F32, tag="ropeB")
    nc.vector.tensor_tensor(tD[:], ps0, sin_ap, ALU.mult)
    o2 = pool.tile([P, 512], F32R, tag="ropeO")
    nc.vector.tensor_tensor(o2[:], tC[:], tD[:], ALU.add)
    nc.sync.dma_start(dst1, o2[:])


def _emit_body(nc, tc, a, slot_L, slot_P):
    # ---------------- Phase 1: QKV projection (+rope for Q,K) --------------
    with tc.tile_pool(name="xt_pool", bufs=1) as xt_pool:
        xT_d = [None] * DC

        # --- K projection: out kT[c(part), t] ; rope pairs (c, c+128) ---
        with (
            tc.tile_pool(name="ktrig", bufs=1) as ktp,
            tc.tile_pool(name="kproj", bufs=2) as kp,
            tc.tile_pool(name="kpsum", bufs=1, space="PSUM") as kps,
            tc.tile_pool(name="krope", bufs=2) as krp,
        ):
            # DMA emission order sets service order: first wk, then the xT
            # stream (K-proj d-loop paces with it), trig tables last.
            wk0 = kp.tile([P, DC, 2 * P], F32R, tag="wk", name="wk0")
            nc.sync.dma_start(wk0[:], a["wk_r"][:, :, 0:2 * P])
            for d in range(DC):
                t_ = xt_pool.tile([P, T], F32R, tag=f"xT{d}", name=f"xT{d}")
                nc.sync.dma_start(t_[:], a["xT_r"][:, d, :])
                xT_d[d] = t_
            cosk_sb = ktp.tile([P, T], F32)
            nc.sync.dma_start(cosk_sb[:], a["cos_k"][:])
            sink_sb = ktp.tile([P, T], F32)
            nc.sync.dma_start(sink_sb[:], a["sin_k"][:])
            for cp in range(NKV * HD // (2 * P)):  # 4 feature pairs (one per kv head)
                if cp == 0:
                    wk_sb = wk0
                else:
                    wk_sb = kp.tile([P, DC, 2 * P], F32R, tag="wk")
                    nc.sync.dma_start(
                        wk_sb[:], a["wk_r"][:, :, 2 * P * cp:2 * P * (cp + 1)])
                for tg in range(2):
                    ps = [[kps.tile([P, 512], F32, tag=f"kps{i}{t}", name=f"kps{i}{t}",
                                    bufs=2)
                           for t in range(2)] for i in range(2)]
                    for d in range(DC):
                        for i in range(2):
                            for t in range(2):
                                tt = 2 * tg + t
                                nc.tensor.matmul(
                                    ps[i][t][:], wk_sb[:, d, i * P:(i + 1) * P],
                                    xT_d[d][:, 512 * tt:512 * (tt + 1)],
                                    start=(d == 0), stop=(d == DC - 1))
                    for t in range(2):
                        tt = 2 * tg + t
                        ts_ = slice(512 * tt, 512 * (tt + 1))
                        _rope_pair(nc, krp, ps[0][t][:], ps[1][t][:],
                                   cosk_sb[:, ts_], sink_sb[:, ts_],
                                   a["kT_sr"][:, 2 * cp, ts_],
                                   a["kT_sr"][:, 2 * cp + 1, ts_])

        # --- V projection: out v[t(part), c] natural ---
        with (
            tc.tile_pool(name="wv_pool", bufs=1) as wvp,
            tc.tile_pool(name="vout", bufs=3) as vp,
            tc.tile_pool(name="vpsum", bufs=1, space="PSUM") as vps,
        ):
            for ch in range(2):  # c halves of 512
                wv_d = []
                for d in range(DC):
                    t_ = wvp.tile([P, 512], F32R, tag=f"wv{d}", name=f"wv{d}")
                    nc.sync.dma_start(t_[:], a["wv_r"][:, d, 512 * ch:512 * (ch + 1)])
                    wv_d.append(t_)
                for tg in range(4):  # groups of 4 t-chunks
                    ps = [vps.tile([P, 512], F32, tag=f"vps{t}", name=f"vps{t}",
                                   bufs=2)
                          for t in range(4)]
                    for d in range(DC):
                        for t in range(4):
                            tc_i = tg * 4 + t
                            nc.tensor.matmul(
                                ps[t][:], xT_d[d][:, P * tc_i:P * (tc_i + 1)],
                                wv_d[d][:],
                                start=(d == 0), stop=(d == DC - 1))
                    for t in range(4):
                        tc_i = tg * 4 + t
                        vo = vp.tile([P, 512], F32R, tag="vo")
                        nc.vector.tensor_copy(vo[:], ps[t][:])
                        nc.sync.dma_start(
                            a["v_sr"][:, tc_i, 512 * ch:512 * (ch + 1)], vo[:])

    # --- Q projection (xT freed): out qT[c(part), t-slot], rope ---
    with (
        tc.tile_pool(name="qtrig", bufs=1) as qtp,
        tc.tile_pool(name="qproj", bufs=2) as qp,
        tc.tile_pool(name="qpsum", bufs=1, space="PSUM") as qps,
        tc.tile_pool(name="qrope", bufs=2) as qrp,
    ):
        xq_t = [[None] * DC, [None] * DC]
        # th=1 stream emitted first: th=0 was prefetched during V-proj, so cp0
        # th0 computes while th1 streams in behind it.
        for th in (0, 1):
            for d in range(DC):
                t_ = qp.tile([P, 512], F32R, tag=f"xq{th}_{d}", name=f"xq{th}_{d}")
                nc.sync.dma_start(t_[:], a["xqT_r"][:, d, 512 * th:512 * (th + 1)])
                xq_t[th][d] = t_
        cosq_sb = qtp.tile([P, NQ], F32)
        nc.sync.dma_start(cosq_sb[:], a["cos_q"][:])
        sinq_sb = qtp.tile([P, NQ], F32)
        nc.sync.dma_start(sinq_sb[:], a["sin_q"][:])
        # cpair (= q head) outer so head 0's qT rows land first for phase 2
        for cp in range(NH * HD // (2 * P)):
            wq_sb = qp.tile([P, DC, 2 * P], F32R, tag="wq")
            nc.sync.dma_start(wq_sb[:], a["wq_r"][:, :, 2 * P * cp:2 * P * (cp + 1)])
            for th in range(2):
                ts_ = slice(512 * th, 512 * (th + 1))
                ps = [qps.tile([P, 512], F32, tag=f"qps{i}", name=f"qps{i}", bufs=2)
                      for i in range(2)]
                for d in range(DC):
                    for i in range(2):
                        nc.tensor.matmul(
                            ps[i][:], wq_sb[:, d, i * P:(i + 1) * P],
                            xq_t[th][d][:],
                            start=(d == 0), stop=(d == DC - 1))
                _rope_pair(nc, qrp, ps[0][:], ps[1][:],
                           cosq_sb[:, ts_], sinq_sb[:, ts_],
                           a["qT_sr"][:, 2 * cp, ts_],
                           a["qT_sr"][:, 2 * cp + 1, ts_])

    # ---------------- Phase 2: attention ------------------------------------
    with tc.tile_pool(name="enct_pool", bufs=1) as enct_pool:
        encT = enct_pool.tile([P, DC, NQ], F32R)  # [feat_chunk, slot*128]

        with (
            tc.tile_pool(name="attn_const", bufs=1) as acp,
            tc.tile_pool(name="kv_pool", bufs=1) as kvp,
            tc.tile_pool(name="sp_pool", bufs=2) as spp,
            tc.tile_pool(name="small_pool", bufs=4) as smp,
            tc.tile_pool(name="spsum", bufs=2, space="PSUM") as sps,
            tc.tile_pool(name="tpsum", bufs=2, space="PSUM") as tps,
            tc.tile_pool(name="epsum", bufs=2, space="PSUM") as eps,
        ):
            ident = acp.tile([P, P], F32)
            make_identity(nc, ident[:])

            for k in range(NKV):
                kT_sb = kvp.tile([P, 2, T], F32R, tag="kT", bufs=2)
                nc.sync.dma_start(kT_sb[:], a["kT_sr"][:, 2 * k:2 * k + 2, :])
                v_sb = kvp.tile([P, NBLK, HD], F32R, tag="v", bufs=2)
                nc.sync.dma_start(v_sb[:], a["v_sr"][:, :, HD * k:HD * (k + 1)])
                for g in range(G):
                    h = k * G + g
                    qT_sb = kvp.tile([P, 2, NQ], F32R, tag="qT", bufs=2)
                    nc.sync.dma_start(qT_sb[:], a["qT_sr"][:, 2 * h:2 * h + 2, :])
                    for s in range(NSLOT):
                        L = slot_L[s]
                        W = L * P  # kv extent in columns
                        W0 = slot_P[s] * P  # all-True prefix needs no mask add
                        tsl = slice(P * s, P * (s + 1))
                        # S = tanh(QK^T/50) into S_sb
                        S_sb = spp.tile([P, T], F32, tag="S", bufs=1)
                        if W0 < W:
                            msk = spp.tile([P, T], BF16, tag="msk")
                            nc.sync.dma_start(
                                msk[:, :W - W0], a["mask_r"][:, s, W0:W])
                        nchunk = (W + 511) // 512
                        for sch in range(nchunk):
                            w0 = 512 * sch
                            w1 = min(W, w0 + 512)
                            ps_s = sps.tile([P, 512], F32, tag="ps_s")
                            for i in range(2):
                                nc.tensor.matmul(
                                    ps_s[:, :w1 - w0], qT_sb[:, i, tsl],
                                    kT_sb[:, i, w0:w1],
                                    start=(i == 0), stop=(i == 1))
                            nc.scalar.activation(
                                S_sb[:, w0:w1], ps_s[:, :w1 - w0],
                                ACTF.Tanh, scale=1.0 / SOFT_CAP)
                        # mask add only over the non-fully-active tail blocks
                        # (prefix blocks are all-True per the host's mask scan)
                        if W0 < W:
                            nc.vector.tensor_tensor(
                                S_sb[:, W0:W], S_sb[:, W0:W], msk[:, :W - W0],
                                ALU.add)
                        mx = smp.tile([P, 1], F32, tag="mx")
                        nc.vector.tensor_reduce(mx[:], S_sb[:, :W], AX.X, ALU.max)
                        nmx = smp.tile([P, 1], F32, tag="nmx")
                        nc.vector.tensor_scalar_mul(nmx[:], mx[:], -EXP_SCALE)
                        P_sb = spp.tile([P, T], F32, tag="P")
                        den = smp.tile([P, 1], F32, tag="den")
                        nc.scalar.activation(
                            P_sb[:, :W], S_sb[:, :W], ACTF.Exp,
                            scale=EXP_SCALE, bias=nmx[:], accum_out=den[:])
                        rcp = smp.tile([P, 1], F32, tag="rcp")
                        nc.vector.reciprocal(rcp[:], den[:])
                        # transpose P 128x128 blocks
                        PT_sb = spp.tile([P, NBLK, P], F32R, tag="PT")
                        for l in range(L):
                            ps_t = tps.tile([P, P], F32, tag="ps_t")
                            nc.tensor.transpose(
                                ps_t[:], P_sb[:, P * l:P * (l + 1)], ident[:])
                            nc.vector.tensor_copy(PT_sb[:, l, :], ps_t[:])
                        # enc = P @ V  [t, 256]
                        ps_e = eps.tile([P, HD], F32, tag="ps_e")
                        for l in range(L):
                            nc.tensor.matmul(
                                ps_e[:], PT_sb[:, l, :], v_sb[:, l, :],
                                start=(l == 0), stop=(l == L - 1))
                        enc_sb = smp.tile([P, HD], F32, tag="enc")
                        nc.vector.tensor_scalar_mul(enc_sb[:], ps_e[:], rcp[:])
                        # transpose enc into encT
                        for i in range(2):
                            ps_et = tps.tile([P, P], F32, tag="ps_et")
                            nc.tensor.transpose(
                                ps_et[:], enc_sb[:, P * i:P * (i + 1)], ident[:])
                            nc.vector.tensor_copy(encT[:, 2 * h + i, tsl], ps_et[:])

        # ---------------- Phase 3: out projection ---------------------------
        with (
            tc.tile_pool(name="oproj", bufs=2) as op,
            tc.tile_pool(name="opsum", bufs=2, space="PSUM") as ops,
        ):
            for dq in range(4):  # dout quarters of 512
                wo_d = []
                for d in range(DC):
                    t_ = op.tile([P, 512], F32R, tag=f"wo{d}", name=f"wo{d}", bufs=2)
                    nc.sync.dma_start(t_[:], a["wo_r"][:, d, 512 * dq:512 * (dq + 1)])
                    wo_d.append(t_)
                for s in range(NSLOT):
                    tsl = slice(P * s, P * (s + 1))
                    ps_o = ops.tile([P, 512], F32, tag="ps_o")
                    for fc in range(DC):
                        nc.tensor.matmul(
                            ps_o[:], encT[:, fc, tsl],
                            wo_d[fc][:],
                            start=(fc == 0), stop=(fc == DC - 1))
                    o_sb = op.tile([P, 512], F32, tag="o_sb")
                    nc.scalar.copy(o_sb[:], ps_o[:])
                    nc.sync.dma_start(
                        a["out_r"][:, s, 512 * dq:512 * (dq + 1)], o_sb[:])


# ======================= host-side glue ====================================

def plan_shards(attn_mask):
    """Returns (assign, slot_L): assign[core] = list of 8 block ids (slot order),
    slot_L[s] = kv extent in blocks (even)."""
    am = np.asarray(attn_mask)
    need = np.zeros((B, NBLK), dtype=np.int64)  # in blocks
    for b in range(B):
        m = am[b, 0]  # [T, T] bool
        anyrow = m.any(axis=1)
        rev = m[:, ::-1]
        last = np.where(anyrow, T - rev.argmax(axis=1), 1)  # cols needed per row
        for i in range(NBLK):
            need[b, i] = int(np.ceil(last[P * i:P * (i + 1)].max() / P))
    assign = []
    for b in range(B):
        order = sorted(range(NBLK), key=lambda i: (need[b, i], i))
        assign.append([order[j] for j in range(0, NBLK, 2)])   # even core
        assign.append([order[j] for j in range(1, NBLK, 2)])   # odd core
    slot_L = []
    for s in range(NSLOT):
        L = max(need[c // 2, assign[c][s]] for c in range(8))
        L = max(2, int(np.ceil(L / 2) * 2))
        slot_L.append(L)
    # leading kv-blocks that are all-True for every core's slot-s rows
    slot_P = []
    for s in range(NSLOT):
        p_min = slot_L[s]
        for c in range(8):
            rows = am[c // 2, 0][P * assign[c][s]:P * (assign[c][s] + 1)]
            p = 0
            while p < p_min and rows[:, P * p:P * (p + 1)].all():
                p += 1
            p_min = min(p_min, p)
        slot_P.append(p_min)
    return assign, slot_L, slot_P


def make_in_maps(x, positions, attn_mask, w_qkv, w_out, assign):
    x = np.asarray(x, dtype=np.float32)
    positions = np.asarray(positions)
    am = np.asarray(attn_mask)
    w_qkv = np.ascontiguousarray(np.asarray(w_qkv, dtype=np.float32))
    w_out = np.ascontiguousarray(np.asarray(w_out, dtype=np.float32))
    wq = np.ascontiguousarray(w_qkv[:, :NH * HD])
    wk = np.ascontiguousarray(w_qkv[:, NH * HD:(NH + NKV) * HD])
    wv = np.ascontiguousarray(w_qkv[:, (NH + NKV) * HD:])
    frac = 2.0 * np.arange(HD // 2, dtype=np.float32) / HD
    inv_ts = (ROPE_BASE ** frac).astype(np.float32) ** -1  # [128]
    in_maps = []
    xT_cache = {}
    for c in range(8):
        b = c // 2
        blocks = assign[c]
        rows = np.concatenate([np.arange(P * i, P * (i + 1)) for i in blocks])
        if b not in xT_cache:
            xT_cache[b] = np.ascontiguousarray(x[b].T)
        xT = xT_cache[b]
        xqT = np.ascontiguousarray(xT[:, rows])
        pos_b = positions[b].astype(np.float32)
        sin_k = np.sin(inv_ts[:, None] * pos_b[None, :]).astype(np.float32)
        cos_k = np.cos(inv_ts[:, None] * pos_b[None, :]).astype(np.float32)
        pos_q = pos_b[rows]
        sin_q = np.sin(inv_ts[:, None] * pos_q[None, :]).astype(np.float32)
        cos_q = np.cos(inv_ts[:, None] * pos_q[None, :]).astype(np.float32)
        mneg = np.where(am[b, 0][rows], np.float32(0.0), np.float32(MASK_NEG))
        in_maps.append(dict(
            xT=xT, xqT=xqT, wq=wq, wk=wk, wv=wv, wo=w_out,
            cos_k=cos_k, sin_k=sin_k, cos_q=cos_q, sin_q=sin_q,
            maskneg=mneg.astype(ml_dtypes.bfloat16)))
    return in_maps


def assemble(results, assign):
    out = np.empty((B, T, D), dtype=np.float32)
    for c in range(8):
        b = c // 2
        o = results[c]["out"]  # [NQ, D]
        for s, blk in enumerate(assign[c]):
            out[b, P * blk:P * (blk + 1), :] = o[P * s:P * (s + 1), :]
    return out


# ======================= entry point =======================================

_NC_CACHE = {}


def kernel(x, positions, attn_mask, w_qkv, w_out):
    from concourse.bass_utils import run_bass_kernel_spmd
    assign, slot_L, slot_P = plan_shards(attn_mask)
    key = (tuple(slot_L), tuple(slot_P))
    if key not in _NC_CACHE:
        _NC_CACHE[key] = build_nc(slot_L, slot_P)
    nc = _NC_CACHE[key]
    in_maps = make_in_maps(x, positions, attn_mask, w_qkv, w_out, assign)
    res = run_bass_kernel_spmd(nc, in_maps, list(range(8)), trace=False)
    return assemble(res.results, assign)

